# revision 4
# baseline (speedup 1.0000x reference)
"""Trainium2 Bass kernel for nn_MultiHeadAttn (B=4, S=2048, D=1024, H=16).

Sharding: 8 cores = 4 batches x 2 head-groups (tensor-parallel over heads).
Each core computes one batch's attention for 8 of 16 heads (512 of 1024
feature dims) and a partial output projection; the host sums the two
head-group partials per batch (the "all-reduce" of row-parallel Wo).

Device dataflow (all matmuls in float32r: full PE rate, ~1.5e-4 rel err):
  - Host pre-transposes activations (q/k/v -> [D, S]) and weight slices, so
    the kernel needs no on-device transposes.
  - QT/KT computed feature-major [512, 2048]; V computed token-major with an
    interleaved ones column per head ([128, 8*65] tiles) so the attn@V
    matmul (M=65) also produces the softmax row-sums.
  - Scores computed transposed S^T[k,q] with 2-head row-tiled matmuls
    (K=64 pairs packed at tile_position (0,0)/(64,0)).
  - softmax without max-subtraction (scores/8 ~ N(0,1), exp is safe);
    exp on ScalarE with scale=1/8 fused; division via DVE recip + K=1
    ones-matmul partition-broadcast + DVE multiply.
  - Output projection consumes X^T directly; bv/bo folded into a single
    host-precomputed effective bias.
"""
import numpy as np

B, S, D = 4, 2048, 1024
H = 16
DK = 64
G = 2              # head groups (tensor-parallel factor)
DL = D // G        # 512 local feature dims per core
NHL = H // G       # 8 local heads
NJ = NHL // 2      # 4 head pairs
NT = S // 512      # 4 token tiles of 512
NKC = S // 128     # 16 k-token chunks of 128
NDC = D // 128     # 8 d_in chunks
NM = DL // 128     # 4 local out chunks
NMO = D // 128     # 8 output d chunks

_CACHED = {}


def _build_nc():
    import concourse.bass as bass
    import concourse.tile as tile
    from concourse import bacc, mybir

    FP32 = mybir.dt.float32
    FP32R = mybir.dt.float32r
    AF = mybir.ActivationFunctionType
    ts = bass.ts

    nc = bacc.Bacc(None, target_bir_lowering=False, debug=False)

    qT_d = nc.dram_tensor("qT", [D, S], FP32, kind="ExternalInput")
    kT_d = nc.dram_tensor("kT", [D, S], FP32, kind="ExternalInput")
    vT_d = nc.dram_tensor("vT", [D, S], FP32, kind="ExternalInput")
    wqT_d = nc.dram_tensor("wqT", [D, DL], FP32, kind="ExternalInput")
    wkT_d = nc.dram_tensor("wkT", [D, DL], FP32, kind="ExternalInput")
    wvT_d = nc.dram_tensor("wvT", [D, DL], FP32, kind="ExternalInput")
    woT_d = nc.dram_tensor("woT", [DL, D], FP32, kind="ExternalInput")
    bq_d = nc.dram_tensor("bq", [NM, 128, 1], FP32, kind="ExternalInput")
    bk_d = nc.dram_tensor("bk", [NM, 128, 1], FP32, kind="ExternalInput")
    bo_d = nc.dram_tensor("bo", [NMO, 128, 1], FP32, kind="ExternalInput")
    out_d = nc.dram_tensor("outT", [D, S], FP32, kind="ExternalOutput")

    with tile.TileContext(nc) as tc:
        with (
            tc.tile_pool(name="const", bufs=1) as const,
            tc.tile_pool(name="wflat", bufs=9) as wflat,
            tc.tile_pool(name="wop", bufs=4) as wop,
            tc.tile_pool(name="qkwin", bufs=8) as qkwin,
            tc.tile_pool(name="vtwin", bufs=4) as vtwin,
            tc.tile_pool(name="big", bufs=1) as big,
            tc.tile_pool(name="vaug", bufs=1) as vaug,
            tc.tile_pool(name="ppool", bufs=2) as ppool,
            tc.tile_pool(name="small", bufs=2) as small,
            tc.tile_pool(name="outst", bufs=2) as outst,
            tc.tile_pool(name="ps_mm", bufs=2, space="PSUM") as ps_mm,
            tc.tile_pool(name="ps_s", bufs=2, space="PSUM") as ps_s,
            tc.tile_pool(name="ps_y", bufs=2, space="PSUM") as ps_y,
        ):
            # ---- constants
            ones_f = const.tile([1, 64], FP32, name="ones_f")
            ones_r = const.tile([1, 64], FP32R, name="ones_r")
            nc.vector.memset(ones_f[:], 1.0)
            nc.vector.tensor_copy(ones_r[:], ones_f[:])
            onescols = const.tile([128, NHL, 1], FP32, name="onescols")
            nc.vector.memset(onescols[:], 1.0)
            bq_sb, bk_sb, bo_sb = [], [], []
            for m in range(NM):
                t_ = const.tile([128, 1], FP32, name=f"bq{m}")
                nc.sync.dma_start(t_[:], bq_d[m])
                bq_sb.append(t_)
                t_ = const.tile([128, 1], FP32, name=f"bk{m}")
                nc.sync.dma_start(t_[:], bk_d[m])
                bk_sb.append(t_)
            for m in range(NMO):
                t_ = const.tile([128, 1], FP32, name=f"bo{m}")
                nc.sync.dma_start(t_[:], bo_d[m])
                bo_sb.append(t_)

            # ---- weights (fp32r via casting SWDGE DMA)
            wq_sb, wk_sb, wv_sb = [], [], []
            for kc in range(NDC):
                t_ = wflat.tile([128, DL], FP32R, tag="w", name=f"wq{kc}")
                nc.gpsimd.dma_start(t_[:], wqT_d[ts(kc, 128), :])
                wq_sb.append(t_)
            for kc in range(NDC):
                t_ = wflat.tile([128, DL], FP32R, tag="w", name=f"wk{kc}")
                nc.gpsimd.dma_start(t_[:], wkT_d[ts(kc, 128), :])
                wk_sb.append(t_)
            for kc in range(NDC):
                t_ = wflat.tile([128, DL], FP32R, tag="w", name=f"wv{kc}")
                nc.gpsimd.dma_start(t_[:], wvT_d[ts(kc, 128), :])
                wv_sb.append(t_)
            wo_sb = []
            for jc in range(NJ):
                t_ = wop.tile([128, D], FP32R, tag="wo", name=f"wo{jc}")
                nc.gpsimd.dma_start(t_[:], woT_d[ts(jc, 128), :])
                wo_sb.append(t_)

            # ---- resident activation tiles
            QT = [big.tile([128, S], FP32R, name=f"QT{m}") for m in range(NM)]
            KT = [big.tile([128, S], FP32R, name=f"KT{m}") for m in range(NM)]
            X = [big.tile([128, S], FP32R, name=f"X{j}") for j in range(NJ)]
            VA = [vaug.tile([128, NHL * 65], FP32R, name=f"va{c}")
                  for c in range(NKC)]

            # ---- phase A1/A2: QT, KT projections (feature-major)
            for (src_d, w_sb, b_sb, dst) in (
                (qT_d, wq_sb, bq_sb, QT),
                (kT_d, wk_sb, bk_sb, KT),
            ):
                for t in range(NT):
                    win = []
                    for kc in range(NDC):
                        w_ = qkwin.tile([128, 512], FP32R, tag="win",
                                        name=f"win{kc}")
                        nc.gpsimd.dma_start(
                            w_[:], src_d[ts(kc, 128), ts(t, 512)])
                        win.append(w_)
                    for m in range(NM):
                        ps = ps_mm.tile([128, 512], FP32, tag="mm", name="psA")
                        for kc in range(NDC):
                            nc.tensor.matmul(
                                ps[:], w_sb[kc][:, ts(m, 128)], win[kc][:],
                                start=(kc == 0), stop=(kc == NDC - 1))
                        nc.vector.tensor_scalar_add(
                            dst[m][:, ts(t, 512)], ps[:], b_sb[m][:])

            # ---- phase A3: V projection (token-major, ones-augmented)
            va_view = [va[:].rearrange("p (h c) -> p h c", c=65) for va in VA]
            for c in range(NKC):
                ps = ps_mm.tile([128, 512], FP32, tag="mm", name="psV")
                for kc in range(NDC):
                    vt = vtwin.tile([128, 128], FP32R, tag="vt", name="vt")
                    nc.gpsimd.dma_start(
                        vt[:], vT_d[ts(kc, 128), ts(c, 128)])
                    nc.tensor.matmul(ps[:], vt[:], wv_sb[kc][:],
                                     start=(kc == 0), stop=(kc == NDC - 1))
                ps_v = ps[:].rearrange("p (h c) -> p h c", c=64)
                nc.vector.tensor_copy(va_view[c][:, :, 0:64], ps_v)
                nc.vector.tensor_copy(va_view[c][:, :, 64:65], onescols[:])

            # ---- phase B: attention, head-pair j, q-tile t, k-chunk k
            for j in range(NJ):
                for t in range(NT):
                    ys = [ps_y.tile([65, 512], FP32, tag="y", name=f"y{h}")
                          for h in range(2)]
                    for k in range(NKC):
                        s_ps = ps_s.tile([128, 1024], FP32, tag="s", name="s")
                        nc.tensor.matmul(
                            s_ps[:, 0:512], KT[j][0:64, ts(k, 128)],
                            QT[j][0:64, ts(t, 512)],
                            start=True, stop=True, tile_position=(0, 0))
                        nc.tensor.matmul(
                            s_ps[:, 512:1024], KT[j][64:128, ts(k, 128)],
                            QT[j][64:128, ts(t, 512)],
                            start=True, stop=True, tile_position=(64, 0))
                        p = ppool.tile([128, 1024], FP32R, tag="p", name="p")
                        nc.scalar.activation(p[:], s_ps[:], AF.Exp,
                                             scale=0.125)
                        for h in range(2):
                            nc.tensor.matmul(
                                ys[h][:],
                                VA[k][:, 65 * (2 * j + h):
                                      65 * (2 * j + h) + 65],
                                p[:, 512 * h:512 * (h + 1)],
                                start=(k == 0), stop=(k == NKC - 1))
                    for h in range(2):
                        rr = small.tile([1, 512], FP32, tag="rr", name="rr")
                        nc.vector.reciprocal(rr[:], ys[h][64:65, :])
                        rr_r = small.tile([1, 512], FP32R, tag="rrr",
                                          name="rr_r")
                        nc.vector.tensor_copy(rr_r[:], rr[:])
                        rb_ps = ps_mm.tile([64, 512], FP32, tag="mm",
                                           name="rb")
                        nc.tensor.matmul(rb_ps[:], ones_r[:], rr_r[:],
                                         start=True, stop=True)
                        rb_sb = small.tile([64, 512], FP32, tag="rb",
                                           name="rb_sb")
                        nc.vector.tensor_copy(rb_sb[:], rb_ps[:])
                        nc.vector.tensor_mul(
                            X[j][64 * h:64 * h + 64, ts(t, 512)],
                            ys[h][0:64, :], rb_sb[:])

            # ---- phase C: output projection (partial, host sums pairs)
            for m in range(NMO):
                for t in range(NT):
                    ps = ps_mm.tile([128, 512], FP32, tag="mm", name="psO")
                    for j in range(NJ):
                        nc.tensor.matmul(
                            ps[:], wo_sb[j][:, ts(m, 128)],
                            X[j][:, ts(t, 512)],
                            start=(j == 0), stop=(j == NJ - 1))
                    st = outst.tile([128, 512], FP32, tag="st", name="st")
                    nc.vector.tensor_scalar_add(st[:], ps[:], bo_sb[m][:])
                    nc.sync.dma_start(out_d[ts(m, 128), ts(t, 512)], st[:])

    nc.compile()
    return nc


def _prep_in_maps(q, k, v, Wq, bq, Wk, bk, Wv, bv, Wo, bo):
    in_maps = []
    for core in range(8):
        b, g = divmod(core, G)
        rows = slice(DL * g, DL * (g + 1))
        bo_eff = Wo[:, rows].astype(np.float32) @ bv[rows].astype(np.float32)
        if g == 0:
            bo_eff = bo_eff + bo
        in_maps.append({
            "qT": np.ascontiguousarray(q[b].T),
            "kT": np.ascontiguousarray(k[b].T),
            "vT": np.ascontiguousarray(v[b].T),
            "wqT": np.ascontiguousarray(Wq[rows, :].T),
            "wkT": np.ascontiguousarray(Wk[rows, :].T),
            "wvT": np.ascontiguousarray(Wv[rows, :].T),
            "woT": np.ascontiguousarray(Wo[:, rows].T),
            "bq": np.ascontiguousarray(bq[rows].reshape(NM, 128, 1)),
            "bk": np.ascontiguousarray(bk[rows].reshape(NM, 128, 1)),
            "bo": np.ascontiguousarray(
                bo_eff.astype(np.float32).reshape(NMO, 128, 1)),
        })
    return in_maps


def kernel(q, k, v, mask, Wq, bq, Wk, bk, Wv, bv, Wo, bo, _trace=False):
    from concourse.bass_utils import run_bass_kernel_spmd

    q, k, v = (np.asarray(x, dtype=np.float32) for x in (q, k, v))
    Wq, bq, Wk, bk, Wv, bv, Wo, bo = (
        np.asarray(x, dtype=np.float32)
        for x in (Wq, bq, Wk, bk, Wv, bv, Wo, bo))

    if "nc" not in _CACHED:
        _CACHED["nc"] = _build_nc()
    nc = _CACHED["nc"]

    in_maps = _prep_in_maps(q, k, v, Wq, bq, Wk, bk, Wv, bv, Wo, bo)
    res = run_bass_kernel_spmd(nc, in_maps, list(range(8)), trace=_trace)
    if _trace:
        _CACHED["last_result"] = res

    out = np.empty((B, S, D), dtype=np.float32)
    for b in range(B):
        acc = res.results[2 * b]["outT"] + res.results[2 * b + 1]["outT"]
        out[b] = acc.T
    return out


# revision 5
# speedup vs baseline: 1.1277x; 1.1277x over previous
"""Trainium2 Bass kernel for nn_MultiHeadAttn (B=4, S=2048, D=1024, H=16).

Sharding: 8 cores = 4 batches x 2 head-groups (tensor-parallel over heads).
Each core computes one batch's attention for 8 of 16 heads (512 of 1024
feature dims) and a partial output projection; the host sums the two
head-group partials per batch (the "all-reduce" of row-parallel Wo).

Device dataflow (all matmuls in float32r: full PE rate, ~1.5e-4 rel err):
  - Host pre-transposes activations (q/k/v -> [D, S]) and weight slices, so
    the kernel needs no on-device transposes.
  - QT/KT computed feature-major [512, 2048]; V computed token-major with an
    interleaved ones column per head ([128, 8*65] tiles) so the attn@V
    matmul (M=65) also produces the softmax row-sums.
  - Scores computed transposed S^T[k,q] with 2-head row-tiled matmuls
    (K=64 pairs packed at tile_position (0,0)/(64,0)).
  - softmax without max-subtraction (scores/8 ~ N(0,1), exp is safe);
    exp on ScalarE with scale=1/8 fused; division via DVE recip + K=1
    ones-matmul partition-broadcast + DVE multiply.
  - Output projection consumes X^T directly; bv/bo folded into a single
    host-precomputed effective bias.
"""
import numpy as np

B, S, D = 4, 2048, 1024
H = 16
DK = 64
G = 2              # head groups (tensor-parallel factor)
DL = D // G        # 512 local feature dims per core
NHL = H // G       # 8 local heads
NJ = NHL // 2      # 4 head pairs
NT = S // 512      # 4 token tiles of 512
NKC = S // 128     # 16 k-token chunks of 128
NDC = D // 128     # 8 d_in chunks
NM = DL // 128     # 4 local out chunks
NMO = D // 128     # 8 output d chunks

_CACHED = {}


def _build_nc():
    import concourse.bass as bass
    import concourse.tile as tile
    from concourse import bacc, mybir

    FP32 = mybir.dt.float32
    FP32R = mybir.dt.float32r
    AF = mybir.ActivationFunctionType
    ts = bass.ts

    nc = bacc.Bacc(None, target_bir_lowering=False, debug=False)

    qT_d = nc.dram_tensor("qT", [D, S], FP32, kind="ExternalInput")
    kT_d = nc.dram_tensor("kT", [D, S], FP32, kind="ExternalInput")
    vT_d = nc.dram_tensor("vT", [D, S], FP32, kind="ExternalInput")
    wqT_d = nc.dram_tensor("wqT", [D, DL], FP32, kind="ExternalInput")
    wkT_d = nc.dram_tensor("wkT", [D, DL], FP32, kind="ExternalInput")
    wvT_d = nc.dram_tensor("wvT", [D, DL], FP32, kind="ExternalInput")
    woT_d = nc.dram_tensor("woT", [DL, D], FP32, kind="ExternalInput")
    bq_d = nc.dram_tensor("bq", [NM, 128, 1], FP32, kind="ExternalInput")
    bk_d = nc.dram_tensor("bk", [NM, 128, 1], FP32, kind="ExternalInput")
    bo_d = nc.dram_tensor("bo", [NMO, 128, 1], FP32, kind="ExternalInput")
    out_d = nc.dram_tensor("outT", [D, S], FP32, kind="ExternalOutput")

    with tile.TileContext(nc) as tc:
        with (
            tc.tile_pool(name="const", bufs=1) as const,
            tc.tile_pool(name="wflat", bufs=9) as wflat,
            tc.tile_pool(name="wop", bufs=4) as wop,
            tc.tile_pool(name="qkwin", bufs=8) as qkwin,
            tc.tile_pool(name="vtwin", bufs=4) as vtwin,
            tc.tile_pool(name="big", bufs=1) as big,
            tc.tile_pool(name="vaug", bufs=1) as vaug,
            tc.tile_pool(name="ppool", bufs=2) as ppool,
            tc.tile_pool(name="small", bufs=2) as small,
            tc.tile_pool(name="outst", bufs=2) as outst,
            tc.tile_pool(name="ps_mm", bufs=2, space="PSUM") as ps_mm,
            tc.tile_pool(name="ps_s", bufs=2, space="PSUM") as ps_s,
            tc.tile_pool(name="ps_y", bufs=2, space="PSUM") as ps_y,
        ):
            # ---- constants
            ones_f = const.tile([1, 64], FP32, name="ones_f")
            ones_r = const.tile([1, 64], FP32R, name="ones_r")
            nc.vector.memset(ones_f[:], 1.0)
            nc.vector.tensor_copy(ones_r[:], ones_f[:])
            onescols = const.tile([128, NHL, 1], FP32, name="onescols")
            nc.vector.memset(onescols[:], 1.0)
            bq_sb, bk_sb, bo_sb = [], [], []
            for m in range(NM):
                t_ = const.tile([128, 1], FP32, name=f"bq{m}")
                nc.sync.dma_start(t_[:], bq_d[m])
                bq_sb.append(t_)
                t_ = const.tile([128, 1], FP32, name=f"bk{m}")
                nc.sync.dma_start(t_[:], bk_d[m])
                bk_sb.append(t_)
            for m in range(NMO):
                t_ = const.tile([128, 1], FP32, name=f"bo{m}")
                nc.sync.dma_start(t_[:], bo_d[m])
                bo_sb.append(t_)

            # ---- weights (fp32r via casting SWDGE DMA)
            wq_sb, wk_sb, wv_sb = [], [], []
            for kc in range(NDC):
                t_ = wflat.tile([128, DL], FP32R, tag="w", name=f"wq{kc}")
                nc.gpsimd.dma_start(t_[:], wqT_d[ts(kc, 128), :])
                wq_sb.append(t_)
            for kc in range(NDC):
                t_ = wflat.tile([128, DL], FP32R, tag="w", name=f"wk{kc}")
                nc.gpsimd.dma_start(t_[:], wkT_d[ts(kc, 128), :])
                wk_sb.append(t_)
            for kc in range(NDC):
                t_ = wflat.tile([128, DL], FP32R, tag="w", name=f"wv{kc}")
                nc.gpsimd.dma_start(t_[:], wvT_d[ts(kc, 128), :])
                wv_sb.append(t_)
            wo_sb = []
            for jc in range(NJ):
                t_ = wop.tile([128, D], FP32R, tag="wo", name=f"wo{jc}")
                nc.gpsimd.dma_start(t_[:], woT_d[ts(jc, 128), :])
                wo_sb.append(t_)

            # ---- resident activation tiles
            QT = [big.tile([128, S], FP32R, name=f"QT{m}") for m in range(NM)]
            KT = [big.tile([128, S], FP32R, name=f"KT{m}") for m in range(NM)]
            X = [big.tile([128, S], FP32R, name=f"X{j}") for j in range(NJ)]
            VA = [vaug.tile([128, NHL * 65], FP32R, name=f"va{c}")
                  for c in range(NKC)]

            # ---- phase A1/A2: QT, KT projections (feature-major)
            for (src_d, w_sb, b_sb, dst) in (
                (qT_d, wq_sb, bq_sb, QT),
                (kT_d, wk_sb, bk_sb, KT),
            ):
                for t in range(NT):
                    win = []
                    for kc in range(NDC):
                        w_ = qkwin.tile([128, 512], FP32R, tag="win",
                                        name=f"win{kc}")
                        nc.gpsimd.dma_start(
                            w_[:], src_d[ts(kc, 128), ts(t, 512)])
                        win.append(w_)
                    for m in range(NM):
                        ps = ps_mm.tile([128, 512], FP32, tag="mm", name="psA")
                        for kc in range(NDC):
                            nc.tensor.matmul(
                                ps[:], w_sb[kc][:, ts(m, 128)], win[kc][:],
                                start=(kc == 0), stop=(kc == NDC - 1))
                        nc.vector.tensor_scalar_add(
                            dst[m][:, ts(t, 512)], ps[:], b_sb[m][:])

            # ---- phase A3: V projection (token-major, ones-augmented)
            va_view = [va[:].rearrange("p (h c) -> p h c", c=65) for va in VA]
            for c in range(NKC):
                ps = ps_mm.tile([128, 512], FP32, tag="mm", name="psV")
                for kc in range(NDC):
                    vt = vtwin.tile([128, 128], FP32R, tag="vt", name="vt")
                    nc.gpsimd.dma_start(
                        vt[:], vT_d[ts(kc, 128), ts(c, 128)])
                    nc.tensor.matmul(ps[:], vt[:], wv_sb[kc][:],
                                     start=(kc == 0), stop=(kc == NDC - 1))
                ps_v = ps[:].rearrange("p (h c) -> p h c", c=64)
                nc.vector.tensor_copy(va_view[c][:, :, 0:64], ps_v)
                nc.vector.tensor_copy(va_view[c][:, :, 64:65], onescols[:])

            # ---- phase B: attention, head-pair j, q-tile t, k-chunk k
            for j in range(NJ):
                for t in range(NT):
                    ys = [ps_y.tile([65, 512], FP32, tag="y", name=f"y{h}")
                          for h in range(2)]
                    for k in range(NKC):
                        s_ps = ps_s.tile([128, 1024], FP32, tag="s", name="s")
                        nc.tensor.matmul(
                            s_ps[:, 0:512], KT[j][0:64, ts(k, 128)],
                            QT[j][0:64, ts(t, 512)],
                            start=True, stop=True, tile_position=(0, 0))
                        nc.tensor.matmul(
                            s_ps[:, 512:1024], KT[j][64:128, ts(k, 128)],
                            QT[j][64:128, ts(t, 512)],
                            start=True, stop=True, tile_position=(64, 0))
                        p = ppool.tile([128, 1024], FP32R, tag="p", name="p")
                        nc.scalar.activation(p[:], s_ps[:], AF.Exp,
                                             scale=0.125)
                        for h in range(2):
                            nc.tensor.matmul(
                                ys[h][:],
                                VA[k][:, 65 * (2 * j + h):
                                      65 * (2 * j + h) + 65],
                                p[:, 512 * h:512 * (h + 1)],
                                start=(k == 0), stop=(k == NKC - 1))
                    for h in range(2):
                        rr = small.tile([1, 512], FP32, tag="rr", name="rr")
                        nc.vector.reciprocal(rr[:], ys[h][64:65, :])
                        rr_r = small.tile([1, 512], FP32R, tag="rrr",
                                          name="rr_r")
                        nc.vector.tensor_copy(rr_r[:], rr[:])
                        rb_ps = ps_mm.tile([64, 512], FP32, tag="mm",
                                           name="rb")
                        nc.tensor.matmul(rb_ps[:], ones_r[:], rr_r[:],
                                         start=True, stop=True)
                        rb_sb = small.tile([64, 512], FP32, tag="rb",
                                           name="rb_sb")
                        nc.vector.tensor_copy(rb_sb[:], rb_ps[:])
                        nc.vector.tensor_mul(
                            X[j][64 * h:64 * h + 64, ts(t, 512)],
                            ys[h][0:64, :], rb_sb[:])

            # ---- phase C: output projection (partial, host sums pairs)
            for m in range(NMO):
                for t in range(NT):
                    ps = ps_mm.tile([128, 512], FP32, tag="mm", name="psO")
                    for j in range(NJ):
                        nc.tensor.matmul(
                            ps[:], wo_sb[j][:, ts(m, 128)],
                            X[j][:, ts(t, 512)],
                            start=(j == 0), stop=(j == NJ - 1))
                    st = outst.tile([128, 512], FP32, tag="st", name="st")
                    nc.vector.tensor_scalar_add(st[:], ps[:], bo_sb[m][:])
                    nc.sync.dma_start(out_d[ts(m, 128), ts(t, 512)], st[:])

    nc.compile()
    return nc


def _prep_in_maps(q, k, v, Wq, bq, Wk, bk, Wv, bv, Wo, bo):
    in_maps = []
    for core in range(8):
        b, g = divmod(core, G)
        rows = slice(DL * g, DL * (g + 1))
        bo_eff = Wo[:, rows].astype(np.float32) @ bv[rows].astype(np.float32)
        if g == 0:
            bo_eff = bo_eff + bo
        in_maps.append({
            "qT": np.ascontiguousarray(q[b].T),
            "kT": np.ascontiguousarray(k[b].T),
            "vT": np.ascontiguousarray(v[b].T),
            "wqT": np.ascontiguousarray(Wq[rows, :].T),
            "wkT": np.ascontiguousarray(Wk[rows, :].T),
            "wvT": np.ascontiguousarray(Wv[rows, :].T),
            "woT": np.ascontiguousarray(Wo[:, rows].T),
            "bq": np.ascontiguousarray(bq[rows].reshape(NM, 128, 1)),
            "bk": np.ascontiguousarray(bk[rows].reshape(NM, 128, 1)),
            "bo": np.ascontiguousarray(
                bo_eff.astype(np.float32).reshape(NMO, 128, 1)),
        })
    return in_maps


def kernel(q, k, v, mask, Wq, bq, Wk, bk, Wv, bv, Wo, bo, _trace=False, _tmpdir=None):
    from concourse.bass_utils import run_bass_kernel_spmd

    q, k, v = (np.asarray(x, dtype=np.float32) for x in (q, k, v))
    Wq, bq, Wk, bk, Wv, bv, Wo, bo = (
        np.asarray(x, dtype=np.float32)
        for x in (Wq, bq, Wk, bk, Wv, bv, Wo, bo))

    if "nc" not in _CACHED:
        _CACHED["nc"] = _build_nc()
    nc = _CACHED["nc"]

    in_maps = _prep_in_maps(q, k, v, Wq, bq, Wk, bk, Wv, bv, Wo, bo)
    res = run_bass_kernel_spmd(nc, in_maps, list(range(8)), trace=_trace, tmpdir=_tmpdir)
    if _trace:
        _CACHED["last_result"] = res

    out = np.empty((B, S, D), dtype=np.float32)
    for b in range(B):
        acc = res.results[2 * b]["outT"] + res.results[2 * b + 1]["outT"]
        out[b] = acc.T
    return out


# revision 15
# speedup vs baseline: 1.3434x; 1.1913x over previous
"""Trainium2 Bass kernel for nn_MultiHeadAttn (B=4, S=2048, D=1024, H=16).

Sharding: 8 cores = 4 batches x 2 head-groups (tensor-parallel over heads).
Each core computes one batch's attention for 8 of 16 heads (512 of 1024
feature dims) and a partial output projection; the host sums the two
head-group partials per batch (the "all-reduce" of row-parallel Wo).

Device dataflow (all matmuls in float32r: full PE rate, ~1.5e-4 rel err):
  - Host pre-transposes activations (q/k/v -> [D, S]) and weight slices, so
    the kernel needs no on-device transposes.
  - QT/KT computed feature-major [512, 2048]; V computed token-major with an
    interleaved ones column per head ([128, 8*65] tiles) so the attn@V
    matmul (M=65) also produces the softmax row-sums.
  - Scores computed transposed S^T[k,q] with 2-head row-tiled matmuls
    (K=64 pairs packed at tile_position (0,0)/(64,0)).
  - softmax without max-subtraction (scores/8 ~ N(0,1), exp is safe);
    exp on ScalarE with scale=1/8 fused; division via K=1 ones-matmul
    broadcast of raw row-sums + DVE reciprocal + DVE multiply.
  - The attention k-loop is software-pipelined (next scores matmul emitted
    before the exp-dependent attn@V) and V/Q projections are emitted as
    filler tasks inside the ACT-bound attention phase so the PE never
    idles behind ScalarE.
  - Output projection consumes X^T directly; bv/bo folded into a single
    host-precomputed effective bias; emitted per token-tile right after
    the last head-pair finishes that tile.
"""
import numpy as np

B, S, D = 4, 2048, 1024
H = 16
DK = 64
G = 2              # head groups (tensor-parallel factor)
DL = D // G        # 512 local feature dims per core
NHL = H // G       # 8 local heads
NJ = NHL // 2      # 4 head pairs
NT = S // 512      # 4 token tiles of 512
NKC = S // 128     # 16 k-token chunks of 128
NDC = D // 128     # 8 d_in chunks
NM = DL // 128     # 4 local out chunks
NMO = D // 128     # 8 output d chunks

_CACHED = {}


def _build_nc():
    import concourse.bass as bass
    import concourse.tile as tile
    from concourse import bacc, mybir

    FP32 = mybir.dt.float32
    FP32R = mybir.dt.float32r
    AF = mybir.ActivationFunctionType
    ts = bass.ts

    nc = bacc.Bacc(None, target_bir_lowering=False, debug=False)

    qT_d = nc.dram_tensor("qT", [D, S], FP32, kind="ExternalInput")
    kT_d = nc.dram_tensor("kT", [D, S], FP32, kind="ExternalInput")
    vT_d = nc.dram_tensor("vT", [D, S], FP32, kind="ExternalInput")
    wqT_d = nc.dram_tensor("wqT", [D, DL], FP32, kind="ExternalInput")
    wkT_d = nc.dram_tensor("wkT", [D, DL], FP32, kind="ExternalInput")
    wvT_d = nc.dram_tensor("wvT", [D, DL], FP32, kind="ExternalInput")
    woT_d = nc.dram_tensor("woT", [DL, D], FP32, kind="ExternalInput")
    bq_d = nc.dram_tensor("bq", [NM, 128, 1], FP32, kind="ExternalInput")
    bk_d = nc.dram_tensor("bk", [NM, 128, 1], FP32, kind="ExternalInput")
    bo_d = nc.dram_tensor("bo", [NMO, 128, 1], FP32, kind="ExternalInput")
    out_d = nc.dram_tensor("outT", [D, S], FP32, kind="ExternalOutput")

    with tile.TileContext(nc) as tc:
        with (
            tc.tile_pool(name="const", bufs=1) as const,
            tc.tile_pool(name="wflat", bufs=9) as wflat,
            tc.tile_pool(name="wop", bufs=4) as wop,
            tc.tile_pool(name="qkwin", bufs=8) as qkwin,
            tc.tile_pool(name="wqblk", bufs=4) as wqblk,
            tc.tile_pool(name="vtwin", bufs=4) as vtwin,
            tc.tile_pool(name="big", bufs=1) as big,
            tc.tile_pool(name="vaug", bufs=1) as vaug,
            tc.tile_pool(name="ppool", bufs=3) as ppool,
            tc.tile_pool(name="small", bufs=2) as small,
            tc.tile_pool(name="outst", bufs=2) as outst,
            tc.tile_pool(name="ps_mm", bufs=2, space="PSUM") as ps_mm,
            tc.tile_pool(name="ps_s", bufs=2, space="PSUM") as ps_s,
            tc.tile_pool(name="ps_y", bufs=2, space="PSUM") as ps_y,
        ):
            # ---- constants
            ones_f = const.tile([1, 64], FP32, name="ones_f")
            ones_r = const.tile([1, 64], FP32R, name="ones_r")
            nc.vector.memset(ones_f[:], 1.0)
            nc.vector.tensor_copy(ones_r[:], ones_f[:])
            onescols = const.tile([128, NHL, 1], FP32, name="onescols")
            nc.vector.memset(onescols[:], 1.0)
            bq_sb, bk_sb, bo_sb = [], [], []
            for m in range(NM):
                t_ = const.tile([128, 1], FP32, name=f"bq{m}")
                nc.sync.dma_start(t_[:], bq_d[m])
                bq_sb.append(t_)
                t_ = const.tile([128, 1], FP32, name=f"bk{m}")
                nc.sync.dma_start(t_[:], bk_d[m])
                bk_sb.append(t_)
            for m in range(NMO):
                t_ = const.tile([128, 1], FP32, name=f"bo{m}")
                nc.sync.dma_start(t_[:], bo_d[m])
                bo_sb.append(t_)

            # ---- weights (fp32r via casting SWDGE DMA)
            # wk/wv are resident (wk frees before wv's extra slots are
            # needed); wq is loaded just-in-time per [128,128] block inside
            # q_task to avoid a pool-slot cycle with the filler schedule.
            wk_sb, wv_sb = [], []
            for kc in range(NDC):
                t_ = wflat.tile([128, DL], FP32R, tag="w", name=f"wk{kc}")
                nc.gpsimd.dma_start(t_[:], wkT_d[ts(kc, 128), :])
                wk_sb.append(t_)
            for kc in range(NDC):
                t_ = wflat.tile([128, DL], FP32R, tag="w", name=f"wv{kc}")
                nc.gpsimd.dma_start(t_[:], wvT_d[ts(kc, 128), :])
                wv_sb.append(t_)
            wo_sb = []
            for jc in range(NJ):
                t_ = wop.tile([128, D], FP32R, tag="wo", name=f"wo{jc}")
                nc.gpsimd.dma_start(t_[:], woT_d[ts(jc, 128), :])
                wo_sb.append(t_)

            # ---- resident activation tiles
            QT = [big.tile([128, S], FP32R, name=f"QT{m}") for m in range(NM)]
            KT = [big.tile([128, S], FP32R, name=f"KT{m}") for m in range(NM)]
            X = [big.tile([128, S], FP32R, name=f"X{j}") for j in range(NJ)]
            VA = [vaug.tile([128, NHL * 65], FP32R, name=f"va{c}")
                  for c in range(NKC)]
            va_view = [va[:].rearrange("p (h c) -> p h c", c=65) for va in VA]

            # ---- task emitters -------------------------------------------
            def kt_task(t):
                """Project token-tile t of k (all m-chunks, resident wk)."""
                win = []
                for kc in range(NDC):
                    w_ = qkwin.tile([128, 512], FP32R, tag="win",
                                    name=f"win{kc}")
                    nc.gpsimd.dma_start(w_[:], kT_d[ts(kc, 128), ts(t, 512)])
                    win.append(w_)
                for m in range(NM):
                    ps = ps_mm.tile([128, 512], FP32, tag="mm", name="psA")
                    for kc in range(NDC):
                        nc.tensor.matmul(
                            ps[:], wk_sb[kc][:, ts(m, 128)], win[kc][:],
                            start=(kc == 0), stop=(kc == NDC - 1))
                    nc.vector.tensor_scalar_add(
                        KT[m][:, ts(t, 512)], ps[:], bk_sb[m][:])

            def q_task(t):
                """Project token-tile t of q (all m-chunks, JIT wq blocks)."""
                win = []
                for kc in range(NDC):
                    w_ = qkwin.tile([128, 512], FP32R, tag="win",
                                    name=f"win{kc}")
                    nc.gpsimd.dma_start(w_[:], qT_d[ts(kc, 128), ts(t, 512)])
                    win.append(w_)
                for m in range(NM):
                    ps = ps_mm.tile([128, 512], FP32, tag="mm", name="psA")
                    for kc in range(NDC):
                        wb = wqblk.tile([128, 128], FP32R, tag="wqb",
                                        name="wqb")
                        nc.gpsimd.dma_start(
                            wb[:], wqT_d[ts(kc, 128), ts(m, 128)])
                        nc.tensor.matmul(
                            ps[:], wb[:], win[kc][:],
                            start=(kc == 0), stop=(kc == NDC - 1))
                    nc.vector.tensor_scalar_add(
                        QT[m][:, ts(t, 512)], ps[:], bq_sb[m][:])

            def v_task(c):
                """Project token-chunk c of v into the ones-augmented VA."""
                ps = ps_mm.tile([128, 512], FP32, tag="mm", name="psV")
                for kc in range(NDC):
                    vt = vtwin.tile([128, 128], FP32R, tag="vt", name="vt")
                    nc.gpsimd.dma_start(vt[:], vT_d[ts(kc, 128), ts(c, 128)])
                    nc.tensor.matmul(ps[:], vt[:], wv_sb[kc][:],
                                     start=(kc == 0), stop=(kc == NDC - 1))
                ps_v = ps[:].rearrange("p (h c) -> p h c", c=64)
                nc.vector.tensor_copy(va_view[c][:, :, 0:64], ps_v)
                nc.vector.tensor_copy(va_view[c][:, :, 64:65], onescols[:])

            def out_task(t):
                """Output projection for token-tile t (needs all X_j)."""
                for m in range(NMO):
                    ps = ps_mm.tile([128, 512], FP32, tag="mm", name="psO")
                    for j in range(NJ):
                        nc.tensor.matmul(
                            ps[:], wo_sb[j][:, ts(m, 128)],
                            X[j][:, ts(t, 512)],
                            start=(j == 0), stop=(j == NJ - 1))
                    st = outst.tile([128, 512], FP32, tag="st", name="st")
                    nc.vector.tensor_scalar_add(st[:], ps[:], bo_sb[m][:])
                    nc.sync.dma_start(out_d[ts(m, 128), ts(t, 512)], st[:])

            # ---- filler Q-projection tasks, drained inside the ACT-bound
            # attention phase (or force-emitted just before a phase needs
            # them, so no attention matmul ever precedes its producer).
            q_done = set()

            def ensure_q(t):
                if t in q_done:
                    return
                q_done.add(t)
                q_task(t)

            def pop_filler():
                for t in range(1, NT):
                    if t not in q_done:
                        ensure_q(t)
                        return

            def attn_tile(j, t):
                """Attention for head-pair j, token-tile t; pipelined k-loop
                with filler tasks drained into the PE's exp-wait gaps."""
                ys = [ps_y.tile([65, 512], FP32, tag="y", name=f"y{h}")
                      for h in range(2)]

                def scores(k):
                    s_ps = ps_s.tile([128, 1024], FP32, tag="s", name="s")
                    nc.tensor.matmul(
                        s_ps[:, 0:512], KT[j][0:64, ts(k, 128)],
                        QT[j][0:64, ts(t, 512)],
                        start=True, stop=True, tile_position=(0, 0))
                    nc.tensor.matmul(
                        s_ps[:, 512:1024], KT[j][64:128, ts(k, 128)],
                        QT[j][64:128, ts(t, 512)],
                        start=True, stop=True, tile_position=(64, 0))
                    return s_ps

                first = (j == 0 and t == 0)
                if first:
                    v_task(0)
                s_cur = scores(0)
                for k in range(NKC):
                    p = ppool.tile([128, 1024], FP32R, tag="p", name="p")
                    nc.scalar.activation(p[:], s_cur[:], AF.Exp, scale=0.125)
                    if k + 1 < NKC:
                        s_cur = scores(k + 1)
                    if first and k + 1 < NKC:
                        v_task(k + 1)
                    elif j == 2 and k == 3:
                        pop_filler()
                    for h in range(2):
                        nc.tensor.matmul(
                            ys[h][:],
                            VA[k][:, 65 * (2 * j + h):65 * (2 * j + h) + 65],
                            p[:, 512 * h:512 * (h + 1)],
                            start=(k == 0), stop=(k == NKC - 1))

                for h in range(2):
                    rr_r = small.tile([1, 512], FP32R, tag="rrr", name="rr_r")
                    nc.vector.tensor_copy(rr_r[:], ys[h][64:65, :])
                    rb_ps = ps_mm.tile([64, 512], FP32, tag="mm", name="rb")
                    nc.tensor.matmul(rb_ps[:], ones_r[:], rr_r[:],
                                     start=True, stop=True)
                    ri = small.tile([64, 512], FP32, tag="ri", name="ri")
                    nc.vector.reciprocal(ri[:], rb_ps[:])
                    nc.vector.tensor_mul(
                        X[j][64 * h:64 * h + 64, ts(t, 512)],
                        ys[h][0:64, :], ri[:])

            # ---- emission ------------------------------------------------
            for t in range(NT):
                kt_task(t)
            ensure_q(0)

            for t in range(NT):
                ensure_q(t)
                for j in range(NJ):
                    attn_tile(j, t)
                out_task(t)

    nc.compile()
    return nc


def _prep_in_maps(q, k, v, Wq, bq, Wk, bk, Wv, bv, Wo, bo):
    in_maps = []
    for core in range(8):
        b, g = divmod(core, G)
        rows = slice(DL * g, DL * (g + 1))
        bo_eff = Wo[:, rows].astype(np.float32) @ bv[rows].astype(np.float32)
        if g == 0:
            bo_eff = bo_eff + bo
        in_maps.append({
            "qT": np.ascontiguousarray(q[b].T),
            "kT": np.ascontiguousarray(k[b].T),
            "vT": np.ascontiguousarray(v[b].T),
            "wqT": np.ascontiguousarray(Wq[rows, :].T),
            "wkT": np.ascontiguousarray(Wk[rows, :].T),
            "wvT": np.ascontiguousarray(Wv[rows, :].T),
            "woT": np.ascontiguousarray(Wo[:, rows].T),
            "bq": np.ascontiguousarray(bq[rows].reshape(NM, 128, 1)),
            "bk": np.ascontiguousarray(bk[rows].reshape(NM, 128, 1)),
            "bo": np.ascontiguousarray(
                bo_eff.astype(np.float32).reshape(NMO, 128, 1)),
        })
    return in_maps


def kernel(q, k, v, mask, Wq, bq, Wk, bk, Wv, bv, Wo, bo,
           _trace=False, _tmpdir=None):
    from concourse.bass_utils import run_bass_kernel_spmd

    q, k, v = (np.asarray(x, dtype=np.float32) for x in (q, k, v))
    Wq, bq, Wk, bk, Wv, bv, Wo, bo = (
        np.asarray(x, dtype=np.float32)
        for x in (Wq, bq, Wk, bk, Wv, bv, Wo, bo))

    if "nc" not in _CACHED:
        _CACHED["nc"] = _build_nc()
    nc = _CACHED["nc"]

    in_maps = _prep_in_maps(q, k, v, Wq, bq, Wk, bk, Wv, bv, Wo, bo)
    res = run_bass_kernel_spmd(nc, in_maps, list(range(8)), trace=_trace,
                               tmpdir=_tmpdir)
    if _trace:
        _CACHED["last_result"] = res

    out = np.empty((B, S, D), dtype=np.float32)
    for b in range(B):
        acc = res.results[2 * b]["outT"] + res.results[2 * b + 1]["outT"]
        out[b] = acc.T
    return out


# revision 16
# speedup vs baseline: 1.8502x; 1.3772x over previous
"""Trainium2 Bass kernel for nn_MultiHeadAttn (B=4, S=2048, D=1024, H=16).

Sharding: 8 cores = 4 batches x 2 head-groups (tensor-parallel over heads).
Each core computes one batch's attention for 8 of 16 heads (512 of 1024
feature dims) and a partial output projection; the host sums the two
head-group partials per batch (the "all-reduce" of row-parallel Wo).

Device dataflow (matmuls in fp16 with fp32 PSUM accumulation; fp16 keeps
10 mantissa bits so the end-to-end error stays ~5e-4 while enabling the
fast weight-load path the PE lacks for fp32/fp32r):
  - Host pre-transposes activations (q/k/v -> [D, S]) and weight slices and
    converts them to fp16, so the kernel needs no on-device transposes or
    casting DMAs.
  - QT/KT computed feature-major [512, 2048]; V computed token-major with a
    ones column per head ([128, 8*65] tiles) so the attn@V matmul (M=65)
    also produces the softmax row-sums.
  - Scores computed transposed S^T[k,q] with 2-head row-tiled matmuls
    (K=64 pairs packed at tile_position (0,0)/(64,0)).
  - softmax without max-subtraction (scores/8 ~ N(0,1), exp is safe);
    exp on ScalarE with scale=1/8 fused; division via K=1 ones-matmul
    broadcast of raw row-sums + DVE reciprocal + DVE multiply, deferred
    into the next tile's slack so the PE never waits on it.
  - The attention k-loop is software-pipelined (next scores matmul emitted
    before the exp-dependent attn@V); V and Q projections are emitted as
    filler work inside the ACT-bound attention phase.
  - Output projection consumes X^T directly; bv/bo folded into a single
    host-precomputed effective bias; emitted per token-tile right after
    the last head-pair finishes that tile.
"""
import numpy as np

B, S, D = 4, 2048, 1024
H = 16
DK = 64
G = 2              # head groups (tensor-parallel factor)
DL = D // G        # 512 local feature dims per core
NHL = H // G       # 8 local heads
NJ = NHL // 2      # 4 head pairs
NT = S // 512      # 4 token tiles of 512
NKC = S // 128     # 16 k-token chunks of 128
NDC = D // 128     # 8 d_in chunks
NM = DL // 128     # 4 local out chunks
NMO = D // 128     # 8 output d chunks

_CACHED = {}


def _build_nc():
    import concourse.bass as bass
    import concourse.tile as tile
    from concourse import bacc, mybir

    FP32 = mybir.dt.float32
    FP16 = mybir.dt.float16
    AF = mybir.ActivationFunctionType
    ts = bass.ts

    nc = bacc.Bacc(None, target_bir_lowering=False, debug=False)

    qT_d = nc.dram_tensor("qT", [D, S], FP16, kind="ExternalInput")
    kT_d = nc.dram_tensor("kT", [D, S], FP16, kind="ExternalInput")
    vT_d = nc.dram_tensor("vT", [D, S], FP16, kind="ExternalInput")
    wqT_d = nc.dram_tensor("wqT", [D, DL], FP16, kind="ExternalInput")
    wkT_d = nc.dram_tensor("wkT", [D, DL], FP16, kind="ExternalInput")
    wvT_d = nc.dram_tensor("wvT", [D, DL], FP16, kind="ExternalInput")
    woT_d = nc.dram_tensor("woT", [DL, D], FP16, kind="ExternalInput")
    bq_d = nc.dram_tensor("bq", [NM, 128, 1], FP32, kind="ExternalInput")
    bk_d = nc.dram_tensor("bk", [NM, 128, 1], FP32, kind="ExternalInput")
    bo_d = nc.dram_tensor("bo", [NMO, 128, 1], FP32, kind="ExternalInput")
    out_d = nc.dram_tensor("outT", [D, S], FP32, kind="ExternalOutput")

    with tile.TileContext(nc) as tc:
        with (
            tc.tile_pool(name="const", bufs=1) as const,
            tc.tile_pool(name="wflat", bufs=24) as wflat,
            tc.tile_pool(name="wop", bufs=4) as wop,
            tc.tile_pool(name="qkwin", bufs=10) as qkwin,
            tc.tile_pool(name="vtwin", bufs=6) as vtwin,
            tc.tile_pool(name="big", bufs=1) as big,
            tc.tile_pool(name="vaug", bufs=1) as vaug,
            tc.tile_pool(name="ppool", bufs=4) as ppool,
            tc.tile_pool(name="park", bufs=4) as parkp,
            tc.tile_pool(name="small", bufs=3) as small,
            tc.tile_pool(name="outst", bufs=3) as outst,
            tc.tile_pool(name="ps_mm", bufs=2, space="PSUM") as ps_mm,
            tc.tile_pool(name="ps_s", bufs=2, space="PSUM") as ps_s,
            tc.tile_pool(name="ps_y", bufs=2, space="PSUM") as ps_y,
        ):
            # ---- constants
            ones_h = const.tile([1, 64], FP16, name="ones_h")
            nc.vector.memset(ones_h[:], 1.0)
            onescols = const.tile([128, NHL, 1], FP16, name="onescols")
            nc.vector.memset(onescols[:], 1.0)
            bq_sb, bk_sb, bo_sb = [], [], []
            for m in range(NM):
                t_ = const.tile([128, 1], FP32, name=f"bq{m}")
                nc.sync.dma_start(t_[:], bq_d[m])
                bq_sb.append(t_)
                t_ = const.tile([128, 1], FP32, name=f"bk{m}")
                nc.sync.dma_start(t_[:], bk_d[m])
                bk_sb.append(t_)
            for m in range(NMO):
                t_ = const.tile([128, 1], FP32, name=f"bo{m}")
                nc.sync.dma_start(t_[:], bo_d[m])
                bo_sb.append(t_)

            # ---- weights (all resident, fp16)
            wq_sb, wk_sb, wv_sb, wo_sb = [], [], [], []
            for kc in range(NDC):
                t_ = wflat.tile([128, DL], FP16, tag="w", name=f"wk{kc}")
                nc.sync.dma_start(t_[:], wkT_d[ts(kc, 128), :])
                wk_sb.append(t_)
            for kc in range(NDC):
                t_ = wflat.tile([128, DL], FP16, tag="w", name=f"wq{kc}")
                nc.sync.dma_start(t_[:], wqT_d[ts(kc, 128), :])
                wq_sb.append(t_)
            for kc in range(NDC):
                t_ = wflat.tile([128, DL], FP16, tag="w", name=f"wv{kc}")
                nc.sync.dma_start(t_[:], wvT_d[ts(kc, 128), :])
                wv_sb.append(t_)
            for jc in range(NJ):
                t_ = wop.tile([128, D], FP16, tag="wo", name=f"wo{jc}")
                nc.sync.dma_start(t_[:], woT_d[ts(jc, 128), :])
                wo_sb.append(t_)

            # ---- resident activation tiles (fp16)
            QT = [big.tile([128, S], FP16, name=f"QT{m}") for m in range(NM)]
            KT = [big.tile([128, S], FP16, name=f"KT{m}") for m in range(NM)]
            X = [big.tile([128, S], FP16, name=f"X{j}") for j in range(NJ)]
            VA = [vaug.tile([128, NHL * 65], FP16, name=f"va{c}")
                  for c in range(NKC)]
            va_view = [va[:].rearrange("p (h c) -> p h c", c=65) for va in VA]

            # ---- task emitters -------------------------------------------
            def qk_task(src_d, w_sb, b_sb, dst, t):
                """Project token-tile t of q or k (all m-chunks)."""
                win = []
                for kc in range(NDC):
                    w_ = qkwin.tile([128, 512], FP16, tag="win",
                                    name=f"win{kc}")
                    nc.sync.dma_start(w_[:], src_d[ts(kc, 128), ts(t, 512)])
                    win.append(w_)
                for m in range(NM):
                    ps = ps_mm.tile([128, 512], FP32, tag="mm", name="psA")
                    for kc in range(NDC):
                        nc.tensor.matmul(
                            ps[:], w_sb[kc][:, ts(m, 128)], win[kc][:],
                            start=(kc == 0), stop=(kc == NDC - 1))
                    nc.vector.tensor_scalar_add(
                        dst[m][:, ts(t, 512)], ps[:], b_sb[m][:])

            def v_task(c):
                """Project token-chunk c of v into the ones-augmented VA."""
                ps = ps_mm.tile([128, 512], FP32, tag="mm", name="psV")
                for kc in range(NDC):
                    vt = vtwin.tile([128, 128], FP16, tag="vt", name="vt")
                    nc.sync.dma_start(vt[:], vT_d[ts(kc, 128), ts(c, 128)])
                    nc.tensor.matmul(ps[:], vt[:], wv_sb[kc][:],
                                     start=(kc == 0), stop=(kc == NDC - 1))
                ps_v = ps[:].rearrange("p (h c) -> p h c", c=64)
                nc.vector.tensor_copy(va_view[c][:, :, 0:64], ps_v)
                nc.vector.tensor_copy(va_view[c][:, :, 64:65], onescols[:])

            def out_task(t):
                """Output projection for token-tile t (needs all X_j)."""
                for m in range(NMO):
                    ps = ps_mm.tile([128, 512], FP32, tag="mm", name="psO")
                    for j in range(NJ):
                        nc.tensor.matmul(
                            ps[:], wo_sb[j][:, ts(m, 128)],
                            X[j][:, ts(t, 512)],
                            start=(j == 0), stop=(j == NJ - 1))
                    st = outst.tile([128, 512], FP32, tag="st", name="st")
                    nc.vector.tensor_scalar_add(st[:], ps[:], bo_sb[m][:])
                    nc.sync.dma_start(out_d[ts(m, 128), ts(t, 512)], st[:])

            # ---- filler Q-projection tasks + deferred division work,
            # drained inside the ACT-bound attention phase.
            q_done = set()

            def ensure_q(t):
                if t in q_done:
                    return
                q_done.add(t)
                qk_task(qT_d, wq_sb, bq_sb, QT, t)

            def pop_q_filler():
                for t in range(1, NT):
                    if t not in q_done:
                        ensure_q(t)
                        return

            pending_div = []

            def division(j, t, parks):
                """Normalize parked Y by its rowsum row and write X."""
                for h in range(2):
                    pk = parks[h]
                    rr = small.tile([1, 512], FP16, tag="rr", name="rr")
                    nc.vector.tensor_copy(rr[:], pk[64:65, :])
                    rb_ps = ps_mm.tile([64, 512], FP32, tag="mm", name="rb")
                    nc.tensor.matmul(rb_ps[:], ones_h[:], rr[:],
                                     start=True, stop=True)
                    ri = small.tile([64, 512], FP32, tag="ri", name="ri")
                    nc.vector.reciprocal(ri[:], rb_ps[:])
                    nc.vector.tensor_mul(
                        X[j][64 * h:64 * h + 64, ts(t, 512)],
                        pk[0:64, :], ri[:])

            def flush_div():
                while pending_div:
                    args = pending_div.pop(0)
                    division(*args)

            def attn_tile(j, t):
                """Attention for head-pair j, token-tile t; pipelined k-loop
                with filler work drained into the PE's exp-wait gaps."""
                ys = [ps_y.tile([65, 512], FP32, tag="y", name=f"y{h}")
                      for h in range(2)]

                def scores(k):
                    s_ps = ps_s.tile([128, 1024], FP32, tag="s", name="s")
                    nc.tensor.matmul(
                        s_ps[:, 0:512], KT[j][0:64, ts(k, 128)],
                        QT[j][0:64, ts(t, 512)],
                        start=True, stop=True, tile_position=(0, 0))
                    nc.tensor.matmul(
                        s_ps[:, 512:1024], KT[j][64:128, ts(k, 128)],
                        QT[j][64:128, ts(t, 512)],
                        start=True, stop=True, tile_position=(64, 0))
                    return s_ps

                first = (j == 0 and t == 0)
                if first:
                    v_task(0)
                s_cur = scores(0)
                for k in range(NKC):
                    p = ppool.tile([128, 1024], FP16, tag="p", name="p")
                    nc.scalar.activation(p[:], s_cur[:], AF.Exp, scale=0.125)
                    if k + 1 < NKC:
                        s_cur = scores(k + 1)
                    if first and k + 1 < NKC:
                        v_task(k + 1)
                    elif k == 3:
                        flush_div()
                    elif j == 2 and k == 7:
                        pop_q_filler()
                    for h in range(2):
                        nc.tensor.matmul(
                            ys[h][:],
                            VA[k][:, 65 * (2 * j + h):65 * (2 * j + h) + 65],
                            p[:, 512 * h:512 * (h + 1)],
                            start=(k == 0), stop=(k == NKC - 1))

                # evict Y to SBUF immediately (frees the PSUM bank) and
                # defer the normalization into the next tile's slack
                parks = []
                for h in range(2):
                    pk = parkp.tile([65, 512], FP32, tag="park",
                                    name=f"pk{h}")
                    nc.vector.tensor_copy(pk[:], ys[h][:])
                    parks.append(pk)
                pending_div.append((j, t, parks))

            # ---- emission ------------------------------------------------
            for t in range(NT):
                qk_task(kT_d, wk_sb, bk_sb, KT, t)
            ensure_q(0)

            for t in range(NT):
                ensure_q(t)
                for j in range(NJ):
                    attn_tile(j, t)
                flush_div()
                out_task(t)

    nc.compile()
    return nc


def _prep_in_maps(q, k, v, Wq, bq, Wk, bk, Wv, bv, Wo, bo):
    f16 = np.float16
    in_maps = []
    for core in range(8):
        b, g = divmod(core, G)
        rows = slice(DL * g, DL * (g + 1))
        bo_eff = Wo[:, rows].astype(np.float32) @ bv[rows].astype(np.float32)
        if g == 0:
            bo_eff = bo_eff + bo
        in_maps.append({
            "qT": np.ascontiguousarray(q[b].T.astype(f16)),
            "kT": np.ascontiguousarray(k[b].T.astype(f16)),
            "vT": np.ascontiguousarray(v[b].T.astype(f16)),
            "wqT": np.ascontiguousarray(Wq[rows, :].T.astype(f16)),
            "wkT": np.ascontiguousarray(Wk[rows, :].T.astype(f16)),
            "wvT": np.ascontiguousarray(Wv[rows, :].T.astype(f16)),
            "woT": np.ascontiguousarray(Wo[:, rows].T.astype(f16)),
            "bq": np.ascontiguousarray(bq[rows].reshape(NM, 128, 1)),
            "bk": np.ascontiguousarray(bk[rows].reshape(NM, 128, 1)),
            "bo": np.ascontiguousarray(
                bo_eff.astype(np.float32).reshape(NMO, 128, 1)),
        })
    return in_maps


def kernel(q, k, v, mask, Wq, bq, Wk, bk, Wv, bv, Wo, bo,
           _trace=False, _tmpdir=None):
    from concourse.bass_utils import run_bass_kernel_spmd

    q, k, v = (np.asarray(x, dtype=np.float32) for x in (q, k, v))
    Wq, bq, Wk, bk, Wv, bv, Wo, bo = (
        np.asarray(x, dtype=np.float32)
        for x in (Wq, bq, Wk, bk, Wv, bv, Wo, bo))

    if "nc" not in _CACHED:
        _CACHED["nc"] = _build_nc()
    nc = _CACHED["nc"]

    in_maps = _prep_in_maps(q, k, v, Wq, bq, Wk, bk, Wv, bv, Wo, bo)
    res = run_bass_kernel_spmd(nc, in_maps, list(range(8)), trace=_trace,
                               tmpdir=_tmpdir)
    if _trace:
        _CACHED["last_result"] = res

    out = np.empty((B, S, D), dtype=np.float32)
    for b in range(B):
        acc = res.results[2 * b]["outT"] + res.results[2 * b + 1]["outT"]
        out[b] = acc.T
    return out
